# revision 11
# baseline (speedup 1.0000x reference)
"""Trainium2 Bass kernel for an eager bidirectional attention block.

Reference computation (per batch b):
    q,k,v = (x @ Wq + bq), (x @ Wk + bk), (x @ Wv + bv)   split into 16 heads of 64
    scores = q @ k^T / sqrt(dh);  scores[:, masked_k] = -inf
    out = softmax(scores) @ v;    y = concat_heads(out) @ Wo + bo

Sharding (8 cores): core c owns batch b = c//2 and heads [8*(c%2), 8*(c%2)+8).
Each core computes a partial y_c = sum_{its heads} softmax(..) v_h @ Wo[h-rows, :]
(fp32, [S, D]); the host sums the two partials per batch and adds bo.
No collectives are needed.

Per-core layout (all matmuls bf16, fp32 PSUM accumulate):
  - x is uploaded transposed (x^T [D, S]); QKV projections produce Q^T/K^T
    [dh, S] with 2 heads per 128-partition tile (head A rows 0-63, B 64-127).
  - scores are computed transposed: S^T [k, q] = K^T-stationary @ Q^T.
  - mask is folded into V:  V' = [V * m | m]  (65 columns per head); PV gives
    the unnormalized output and the softmax denominator in one accumulation.
  - 1/sqrt(dh) folded into Wq/bq on the host.

Scheduling (v4) — built from HW microbenchmarks:
  - A K=64 matmul whose row group alternates with its neighbor runs ~free
    (concurrent quadrants); same-row-group back-to-back singles cost ~1us.
    Accumulation chains must stay on ONE row group (mixing row groups inside
    a PSUM accumulation group hangs the device).
  - So: the head pair's scores (rows 0-63 / 64-127) are emitted around the
    PV ops, and each head's PV is SPLIT into key-half chains (lo keys ->
    rows 0-63, hi keys -> rows 64-127) accumulating into separate PSUM
    tiles that are summed during normalization.  Adjacent PE ops then
    always alternate row groups.
  - Both heads' scores land in one [128,1024] PSUM tile (2 banks), so ONE
    wide ACT/DVE op computes exp for the whole kt step (per-op overhead on
    ACT/DVE is several hundred ns; wide tiles amortize it).  exp alternates
    between ACT (true exp) and DVE (Schraudolph int16 bit-trick, ~3% err)
    per kt; Pool (SBUF-only) does the normalization broadcast/multiply.
  - PSUM: mega-score ring 2x[128,1024] (4 banks) + 4 PV accumulators
    [65,512] (4 banks) = 8.  Projections run in PE-only blocks using the
    mega ring (attention passes leave no spare banks).
"""

import numpy as np
import ml_dtypes

BF16 = ml_dtypes.bfloat16

# Full problem dims
B, S, D, H, DH = 4, 2048, 1024, 16, 64
N_CORES = 8
HC = 8  # heads per core

SCH_A = 128.0 / float(np.log(2.0))  # 2^7/ln2
SCH_C = 16256.0 - 4.6               # 127*2^7 - bias correction

# kt indices whose exp runs on DVE (Schraudolph); rest on ACT
DVE_KTS = frozenset((1, 3, 5, 7, 9, 11))


def build_nc(S=S, D=D, HC=HC, dh=DH, num_devices=N_CORES, reps=1, probe=None,
             with_bv=True):
    from concourse import bacc
    import concourse.mybir as mybir
    from concourse.tile import TileContext

    f32 = mybir.dt.float32
    bf16 = mybir.dt.bfloat16
    i16 = mybir.dt.int16
    Exp = mybir.ActivationFunctionType.Exp
    Identity = mybir.ActivationFunctionType.Identity
    Mult = mybir.AluOpType.mult
    Add = mybir.AluOpType.add

    G = HC // 2           # head pairs
    DT = D // 128
    KT = S // 128
    CH = min(512, S)
    HD = HC * dh          # 512
    WOT = HD // 128
    DCH = D // CH
    QHW = min(512, S)     # q-pass width
    NP = S // QHW
    QCX = S // CH

    nc = bacc.Bacc("TRN2", target_bir_lowering=False, debug=False,
                   num_devices=num_devices)

    xT_d = nc.dram_tensor("xT", [DT * QCX * 128, CH], bf16,
                          kind="ExternalInput").ap()
    wq_d = nc.dram_tensor("wq", [D, HD], bf16, kind="ExternalInput").ap()
    wk_d = nc.dram_tensor("wk", [D, HD], bf16, kind="ExternalInput").ap()
    wv_d = nc.dram_tensor("wv", [D, HD], bf16, kind="ExternalInput").ap()
    wo_d = nc.dram_tensor("wo", [HD, D], bf16, kind="ExternalInput").ap()
    mkf_d = nc.dram_tensor("mkf", [128, KT], f32, kind="ExternalInput").ap()
    bq_d = nc.dram_tensor("bqc", [128, G], f32, kind="ExternalInput").ap()
    bk_d = nc.dram_tensor("bkc", [128, G], f32, kind="ExternalInput").ap()
    bv_d = nc.dram_tensor("bvr", [1, HD], bf16, kind="ExternalInput").ap()
    y_d = nc.dram_tensor("y", [S, D], f32, kind="ExternalOutput").ap()

    with TileContext(nc) as tc:
      for _rep in range(reps):
        with tc.tile_pool(name="const", bufs=1) as cp:
            mkf = cp.tile([128, KT], f32, tag="mkf")
            nc.sync.dma_start(mkf[:], mkf_d[:, :])
            bqc = cp.tile([128, G], f32, tag="bqc")
            nc.sync.dma_start(bqc[:], bq_d[:, :])
            bkc = cp.tile([128, G], f32, tag="bkc")
            nc.sync.dma_start(bkc[:], bk_d[:, :])
            bvr = cp.tile([1, HD], bf16, tag="bvr")
            nc.sync.dma_start(bvr[:], bv_d[:, :])
            ones = cp.tile([1, 128], bf16, tag="ones")
            nc.vector.memset(ones[:], 1.0)
            ones8 = cp.tile([128, HC], f32, tag="ones8")
            nc.vector.memset(ones8[:], 1.0)
            wq_s, wk_s, wv_s = [], [], []
            # DMA order: qk0 needs wq+wk+xT; the v-projection block follows
            # and needs wv; wo is needed only at the end.
            for lst, dram, nm in ((wq_s, wq_d, "wq"), (wk_s, wk_d, "wk")):
                for dt in range(DT):
                    t = cp.tile([128, HD], bf16, name=f"{nm}{dt}", tag=f"{nm}{dt}")
                    nc.sync.dma_start(t[:], dram[dt * 128:(dt + 1) * 128, :])
                    lst.append(t)
            xT_s = [cp.tile([128, S], bf16, name=f"xT{dt}", tag=f"xT{dt}")
                    for dt in range(DT)]
            for c in range(QCX):
                for dt in range(DT):
                    r0 = (dt * QCX + c) * 128
                    nc.sync.dma_start(
                        xT_s[dt][:, c * CH:(c + 1) * CH],
                        xT_d[r0:r0 + 128, :])
                if c == 0:
                    for dt in range(DT):
                        t = cp.tile([128, HD], bf16, name=f"wv{dt}",
                                    tag=f"wv{dt}")
                        nc.sync.dma_start(t[:],
                                          wv_d[dt * 128:(dt + 1) * 128, :])
                        wv_s.append(t)
            wo_s = []
            for wt in range(WOT):
                t = cp.tile([128, D], bf16, name=f"wo{wt}", tag=f"wo{wt}")
                nc.sync.dma_start(t[:], wo_d[wt * 128:(wt + 1) * 128, :])
                wo_s.append(t)

            qT = [cp.tile([128, S], bf16, name=f"qT{g}", tag=f"qT{g}")
                  for g in range(G)]
            kT = [cp.tile([128, S], bf16, name=f"kT{g}", tag=f"kT{g}")
                  for g in range(G)]
            vP = [cp.tile([128, HC * (dh + 1)], bf16, name=f"vP{kt}",
                          tag=f"vP{kt}") for kt in range(KT)]
            oT = [cp.tile([128, S], bf16, name=f"oT{g}", tag=f"oT{g}")
                  for g in range(G)]

            for st in range(KT):
                mc = vP[st][:, :].rearrange("p (h c) -> p h c", c=dh + 1)[:, :, dh]
                nc.vector.tensor_scalar(mc, ones8[:], mkf[:, st:st + 1], None,
                                        op0=Mult)

            with tc.tile_pool(name="pssm", bufs=2, space="PSUM") as pssm, \
                 tc.tile_pool(name="pso", bufs=1, space="PSUM") as pso, \
                 tc.tile_pool(name="ptp", bufs=3) as ptp, \
                 tc.tile_pool(name="nrm", bufs=2) as nrm, \
                 tc.tile_pool(name="ysb", bufs=3) as ysb:

                def mega():
                    return pssm.tile([128, 2 * CH], f32, name="sp", tag="sp")

                def v_proj(st):
                    pv = mega()
                    for dt in range(DT):
                        nc.tensor.matmul(
                            pv[:, :HD],
                            lhsT=xT_s[dt][:, st * 128:(st + 1) * 128],
                            rhs=wv_s[dt][:], start=(dt == 0),
                            stop=(not with_bv and dt == DT - 1))
                    if with_bv:
                        nc.tensor.matmul(pv[:, :HD], lhsT=ones[:], rhs=bvr[:],
                                         start=False, stop=True)
                    vdst = vP[st][:, :].rearrange(
                        "p (h c) -> p h c", c=dh + 1)[:, :, 0:dh]
                    vsrc = pv[:, :HD].rearrange(
                        "p (h c) -> p h c", c=dh)[:, :, :]
                    nc.vector.tensor_scalar(vdst, vsrc, mkf[:, st:st + 1],
                                            None, op0=Mult)

                def qk_chunk(g, which, c):
                    dst, w_s, bcol = ((qT, wq_s, bqc) if which == 0
                                      else (kT, wk_s, bkc))
                    pq = mega()
                    for dt in range(DT):
                        nc.tensor.matmul(
                            pq[:, 0:CH],
                            lhsT=w_s[dt][:, g * 128:(g + 1) * 128],
                            rhs=xT_s[dt][:, c * CH:(c + 1) * CH],
                            start=(dt == 0), stop=(dt == DT - 1))
                    nc.scalar.activation(dst[g][:, c * CH:(c + 1) * CH],
                                         pq[:, 0:CH], Identity,
                                         bias=bcol[:, g:g + 1])

                def final_chunk(qt, c):
                    pf = mega()
                    for wt in range(WOT):
                        nc.tensor.matmul(
                            pf[:, 0:CH],
                            lhsT=oT[wt][:, qt * 128:(qt + 1) * 128],
                            rhs=wo_s[wt][:, c * CH:(c + 1) * CH],
                            start=(wt == 0), stop=(wt == WOT - 1))
                    ys = ysb.tile([128, CH], f32, name="ys", tag="ys")
                    nc.vector.tensor_copy(ys[:], pf[:, 0:CH])
                    nc.sync.dma_start(
                        y_d[qt * 128:(qt + 1) * 128,
                            c * CH:(c + 1) * CH], ys[:])

                def attention_pair(g, p):
                    q0 = p * QHW
                    hA, hB = 2 * g, 2 * g + 1
                    ew = (2 * CH) // 4 if probe == "smallexp" else 2 * CH
                    ops = {}
                    pend = None

                    def emit_scoreA(kt, s):
                        nc.tensor.matmul(
                            s[:, 0:CH],
                            lhsT=kT[g][0:64, kt * 128:(kt + 1) * 128],
                            rhs=qT[g][0:64, q0:q0 + QHW],
                            start=True, stop=True)

                    def emit_scoreB(kt, s):
                        nc.tensor.matmul(
                            s[:, CH:2 * CH],
                            lhsT=kT[g][64:128, kt * 128:(kt + 1) * 128],
                            rhs=qT[g][64:128, q0:q0 + QHW],
                            start=True, stop=True)

                    def emit_pv_half(kt, pt_pair, key, h, half):
                        # half 0: keys 0-63 (rows lo); half 1: keys 64-127
                        pt, cast = pt_pair
                        r0, r1 = (0, 64) if half == 0 else (64, 128)
                        col0 = 0 if key == "A" else CH
                        if probe == "smallpv" and kt not in (0, KT - 1):
                            return
                        rhs = pt[r0:r1, col0:col0 + CH]
                        if cast:
                            rhs = rhs.bitcast(bf16)
                        nc.tensor.matmul(
                            ops[key + str(half)][:],
                            lhsT=vP[kt][r0:r1, h * 65:(h + 1) * 65],
                            rhs=rhs,
                            start=(kt == 0), stop=(kt == KT - 1))

                    for kt in range(KT):
                        s = mega()
                        # zig-zag row groups: sA(lo), pvA_hi, pvA_lo, sB(hi),
                        # pvB_lo, pvB_hi  — adjacent ops alternate quadrants
                        emit_scoreA(kt, s)
                        if pend is not None:
                            if kt == 1:
                                for tg, nmh in (("A0", "oAl"), ("A1", "oAh"),
                                                ("B0", "oBl"), ("B1", "oBh")):
                                    ops[tg] = pso.tile([65, QHW], f32,
                                                       name=nmh, tag=nmh)
                            emit_pv_half(kt - 1, pend, "A", hA, 1)
                            emit_pv_half(kt - 1, pend, "A", hA, 0)
                        emit_scoreB(kt, s)
                        if pend is not None:
                            emit_pv_half(kt - 1, pend, "B", hB, 0)
                            emit_pv_half(kt - 1, pend, "B", hB, 1)
                        if kt not in DVE_KTS:
                            pt = ptp.tile([128, 2 * CH], bf16, name="pt",
                                          tag="pta")
                            nc.scalar.activation(pt[:, :ew], s[:, :ew], Exp)
                            pend = (pt, False)
                        else:
                            pt = ptp.tile([128, 2 * CH], i16, name="pti",
                                          tag="ptbi")
                            nc.vector.tensor_scalar(pt[:, :ew], s[:, :ew],
                                                    SCH_A, SCH_C,
                                                    op0=Mult, op1=Add)
                            pend = (pt, True)
                    emit_pv_half(KT - 1, pend, "A", hA, 1)
                    emit_pv_half(KT - 1, pend, "A", hA, 0)
                    emit_pv_half(KT - 1, pend, "B", hB, 0)
                    emit_pv_half(KT - 1, pend, "B", hB, 1)
                    # normalization: merge halves (DVE), recip (DVE),
                    # broadcast + multiply (Pool, SBUF-only)
                    for key, off in (("A", 0), ("B", 64)):
                        stg = nrm.tile([65, QHW], f32, name="stg",
                                       tag=f"stg{key}")
                        # two steps: PSUM->SBUF copy, then SBUF+PSUM add
                        # (one PSUM operand max per DVE/ACT instruction)
                        if key == "A":
                            nc.scalar.activation(stg[:], ops["A0"][:],
                                                 Identity)
                        else:
                            nc.vector.tensor_copy(stg[:], ops["B0"][:])
                        nc.vector.tensor_tensor(stg[:], stg[:],
                                                ops[key + "1"][:], op=Add)
                        rr = nrm.tile([1, QHW], f32, name="rr", tag=f"rr{key}")
                        nc.vector.reciprocal(rr[:], stg[64:65, :])
                        bc = nrm.tile([64, QHW], f32, name="bc", tag=f"bc{key}")
                        nc.gpsimd.partition_broadcast(bc[:], rr[:])
                        nc.gpsimd.tensor_tensor(
                            oT[g][off:off + 64, q0:q0 + QHW],
                            stg[0:64, :], bc[:], op=Mult)

                # ---- schedule: PE-only projection blocks between attention
                # passes (no spare PSUM banks to smear during them) ----
                for c in range(QCX):
                    for which in (0, 1):
                        qk_chunk(0, which, c)
                for st in range(KT):
                    v_proj(st)

                for p in range(NP):
                    for g in range(G):
                        if p == 0 and g < G - 1:
                            attention_pair(g, p)
                            for c in range(QCX):
                                for which in (0, 1):
                                    qk_chunk(g + 1, which, c)
                        else:
                            attention_pair(g, p)
                    if p > 0:
                        for qt in range((QHW * (p - 1)) // 128,
                                        (QHW * p) // 128):
                            for c in range(DCH):
                                final_chunk(qt, c)
                for qt in range((QHW * (NP - 1)) // 128, S // 128):
                    for c in range(DCH):
                        final_chunk(qt, c)

    nc.compile()
    return nc


def pack_xT(xt, S, D):
    """[D, S] -> contiguous [DT*QCX*128, CH] blocks matching build_nc's DMAs."""
    CH = min(512, S)
    DT, QCX = D // 128, S // CH
    return np.ascontiguousarray(
        xt.reshape(DT, 128, QCX, CH).transpose(0, 2, 1, 3)
    ).reshape(DT * QCX * 128, CH)


def host_shard(x, mask, Wq, bq, Wk, bk, Wv, bv, Wo, bo,
               S=S, D=D, HC=HC, dh=DH):
    """Build the 8 per-core input maps (host-side layout prep)."""
    KT = S // 128
    G = HC // 2
    HD = HC * dh
    scale = 1.0 / np.sqrt(dh)
    in_maps = []
    x = np.asarray(x, np.float32)
    mask = np.asarray(mask)
    for c in range(N_CORES):
        b = c // 2
        hs = (c % 2) * HD
        cols = slice(hs, hs + HD)
        m = 1.0 - mask[b].astype(np.float32)
        in_maps.append({
            "xT": pack_xT(np.ascontiguousarray(x[b].T), S, D).astype(BF16),
            "wq": (np.asarray(Wq)[:, cols] * scale).astype(BF16),
            "wk": np.asarray(Wk)[:, cols].astype(BF16),
            "wv": np.asarray(Wv)[:, cols].astype(BF16),
            "wo": np.asarray(Wo)[cols, :].astype(BF16),
            "mkf": np.ascontiguousarray(m.reshape(KT, 128).T),
            "bqc": np.ascontiguousarray(
                (np.asarray(bq, np.float32)[cols] * scale).reshape(G, 128).T),
            "bkc": np.ascontiguousarray(
                np.asarray(bk, np.float32)[cols].reshape(G, 128).T),
            "bvr": np.asarray(bv, np.float32)[cols].reshape(1, HD).astype(BF16),
        })
    return in_maps


def host_gather(results, bo, B=B, S=S, D=D):
    out = np.empty((B, S, D), np.float32)
    bo = np.asarray(bo, np.float32)
    for b in range(B):
        out[b] = results[2 * b]["y"] + results[2 * b + 1]["y"] + bo
    return out


_NC_CACHE = {}


def kernel(x, mask, Wq, bq, Wk, bk, Wv, bv, Wo, bo):
    from concourse.bass_utils import run_bass_kernel_spmd
    with_bv = bool(np.any(np.asarray(bv)))
    if with_bv not in _NC_CACHE:
        _NC_CACHE[with_bv] = build_nc(with_bv=with_bv)
    in_maps = host_shard(x, mask, Wq, bq, Wk, bk, Wv, bv, Wo, bo)
    res = run_bass_kernel_spmd(_NC_CACHE[with_bv], in_maps,
                               core_ids=list(range(N_CORES)))
    return host_gather(res.results, bo)


# revision 12
# speedup vs baseline: 1.4938x; 1.4938x over previous
"""Trainium2 Bass kernel for an eager bidirectional attention block.

Reference computation (per batch b):
    q,k,v = (x @ Wq + bq), (x @ Wk + bk), (x @ Wv + bv)   split into 16 heads of 64
    scores = q @ k^T / sqrt(dh);  scores[:, masked_k] = -inf
    out = softmax(scores) @ v;    y = concat_heads(out) @ Wo + bo

Sharding (8 cores): core c owns batch b = c//2 and heads [8*(c%2), 8*(c%2)+8).
Each core computes a partial y_c = sum_{its heads} softmax(..) v_h @ Wo[h-rows, :]
(fp32, [S, D]); the host sums the two partials per batch and adds bo.
No collectives are needed.

Per-core layout (all matmuls bf16, fp32 PSUM accumulate):
  - x is uploaded transposed (x^T [D, S]); QKV projections produce Q^T/K^T
    [dh, S] with 2 heads per 128-partition tile (head A rows 0-63, B 64-127).
  - scores are computed transposed: S^T [k, q] = K^T-stationary @ Q^T.
  - mask is folded into V:  V' = [V * m | m]  (65 columns per head); PV gives
    the unnormalized output and the softmax denominator in one accumulation.
  - 1/sqrt(dh) folded into Wq/bq on the host.

Scheduling (v5) — informed by HW microbenchmarks:
  - K=64 matmuls whose row groups alternate run concurrently on the PE
    quadrants; same-row-group singles cost ~1us each.  Accumulation groups
    must stay on a single row group (mixing row groups in one group hangs
    the device).
  - Head pairs are processed together: scores A (rows 0-63) / B (64-127)
    into separate 1-bank PSUM tiles, and each head's PV is split into
    key-half chains (lo keys->rows 0-63, hi->64-127) accumulating into 4
    separate [65,512] PSUM tiles, zig-zagged so adjacent PE ops alternate
    row groups.  Halves are summed during normalization.
  - exp per (head, kt) tile: ACT true exp for most A-tiles, DVE Schraudolph
    (int16 bit-trick, ~3%) for B + a few A; normalization merge on DVE,
    reciprocal/broadcast/multiply on Pool (SBUF-only engine).
  - PSUM: 3 score bufs + 4 PV accumulators + 1 projection buf = 8 banks.
    Projection work (QK of the next pair, V, final O) is smeared into
    attention steps so the PE fills the exp-bound slack.
"""

import numpy as np
import ml_dtypes

BF16 = ml_dtypes.bfloat16

# Full problem dims
B, S, D, H, DH = 4, 2048, 1024, 16, 64
N_CORES = 8
HC = 8  # heads per core

SCH_A = 128.0 / float(np.log(2.0))  # 2^7/ln2
SCH_C = 16256.0 - 4.6               # 127*2^7 - bias correction

# A-head kt indices whose exp runs on DVE in sweep passes (B always DVE)
A_DVE_KTS = frozenset((3, 8, 13))


def build_nc(S=S, D=D, HC=HC, dh=DH, num_devices=N_CORES, reps=1, probe=None,
             with_bv=True):
    from concourse import bacc
    import concourse.mybir as mybir
    from concourse.tile import TileContext

    f32 = mybir.dt.float32
    bf16 = mybir.dt.bfloat16
    i16 = mybir.dt.int16
    Exp = mybir.ActivationFunctionType.Exp
    Mult = mybir.AluOpType.mult
    Add = mybir.AluOpType.add

    G = HC // 2           # head pairs
    DT = D // 128
    KT = S // 128
    CH = min(512, S)
    HD = HC * dh          # 512
    WOT = HD // 128
    DCH = D // CH
    QHW = min(512, S)     # q-pass width (1 bank)
    NP = S // QHW
    QCX = S // CH

    nc = bacc.Bacc("TRN2", target_bir_lowering=False, debug=False,
                   num_devices=num_devices)

    xT_d = nc.dram_tensor("xT", [DT * QCX * 128, CH], bf16,
                          kind="ExternalInput").ap()
    wq_d = nc.dram_tensor("wq", [D, HD], bf16, kind="ExternalInput").ap()
    wk_d = nc.dram_tensor("wk", [D, HD], bf16, kind="ExternalInput").ap()
    wv_d = nc.dram_tensor("wv", [D, HD], bf16, kind="ExternalInput").ap()
    wo_d = nc.dram_tensor("wo", [HD, D], bf16, kind="ExternalInput").ap()
    mkf_d = nc.dram_tensor("mkf", [128, KT], f32, kind="ExternalInput").ap()
    bq_d = nc.dram_tensor("bqc", [128, G], f32, kind="ExternalInput").ap()
    bk_d = nc.dram_tensor("bkc", [128, G], f32, kind="ExternalInput").ap()
    bv_d = nc.dram_tensor("bvr", [1, HD], bf16, kind="ExternalInput").ap()
    y_d = nc.dram_tensor("y", [S, D], f32, kind="ExternalOutput").ap()

    with TileContext(nc) as tc:
      for _rep in range(reps):
        with tc.tile_pool(name="const", bufs=1) as cp:
            mkf = cp.tile([128, KT], f32, tag="mkf")
            nc.sync.dma_start(mkf[:], mkf_d[:, :])
            bqc = cp.tile([128, G], f32, tag="bqc")
            nc.sync.dma_start(bqc[:], bq_d[:, :])
            bkc = cp.tile([128, G], f32, tag="bkc")
            nc.sync.dma_start(bkc[:], bk_d[:, :])
            bvr = cp.tile([1, HD], bf16, tag="bvr")
            nc.sync.dma_start(bvr[:], bv_d[:, :])
            ones = cp.tile([1, 128], bf16, tag="ones")
            nc.vector.memset(ones[:], 1.0)
            ones8 = cp.tile([128, HC], f32, tag="ones8")
            nc.vector.memset(ones8[:], 1.0)
            wq_s, wk_s, wv_s = [], [], []
            for lst, dram, nm in ((wq_s, wq_d, "wq"), (wk_s, wk_d, "wk"),
                                  (wv_s, wv_d, "wv")):
                for dt in range(DT):
                    t = cp.tile([128, HD], bf16, name=f"{nm}{dt}", tag=f"{nm}{dt}")
                    nc.sync.dma_start(t[:], dram[dt * 128:(dt + 1) * 128, :])
                    lst.append(t)
            xT_s = [cp.tile([128, S], bf16, name=f"xT{dt}", tag=f"xT{dt}")
                    for dt in range(DT)]
            for c in range(QCX):
                for dt in range(DT):
                    r0 = (dt * QCX + c) * 128
                    nc.sync.dma_start(
                        xT_s[dt][:, c * CH:(c + 1) * CH],
                        xT_d[r0:r0 + 128, :])
            wo_s = []
            for wt in range(WOT):
                t = cp.tile([128, D], bf16, name=f"wo{wt}", tag=f"wo{wt}")
                nc.sync.dma_start(t[:], wo_d[wt * 128:(wt + 1) * 128, :])
                wo_s.append(t)

            qT = [cp.tile([128, S], bf16, name=f"qT{g}", tag=f"qT{g}")
                  for g in range(G)]
            kT = [cp.tile([128, S], bf16, name=f"kT{g}", tag=f"kT{g}")
                  for g in range(G)]
            vP = [cp.tile([128, HC * (dh + 1)], bf16, name=f"vP{kt}",
                          tag=f"vP{kt}") for kt in range(KT)]
            oT = [cp.tile([128, S], bf16, name=f"oT{g}", tag=f"oT{g}")
                  for g in range(G)]

            for st in range(KT):
                mc = vP[st][:, :].rearrange("p (h c) -> p h c", c=dh + 1)[:, :, dh]
                nc.vector.tensor_scalar(mc, ones8[:], mkf[:, st:st + 1], None,
                                        op0=Mult)

            with tc.tile_pool(name="pss", bufs=3, space="PSUM") as pss, \
                 tc.tile_pool(name="pso", bufs=1, space="PSUM") as pso, \
                 tc.tile_pool(name="pp", bufs=1, space="PSUM") as pp, \
                 tc.tile_pool(name="ptp", bufs=3) as ptp, \
                 tc.tile_pool(name="nrm", bufs=2) as nrm, \
                 tc.tile_pool(name="ysb", bufs=3) as ysb:

                def v_proj(st):
                    pv = pp.tile([128, CH], f32, name="pv", tag="pp")
                    for dt in range(DT):
                        nc.tensor.matmul(
                            pv[:, :HD],
                            lhsT=xT_s[dt][:, st * 128:(st + 1) * 128],
                            rhs=wv_s[dt][:], start=(dt == 0),
                            stop=(not with_bv and dt == DT - 1))
                    if with_bv:
                        nc.tensor.matmul(pv[:, :HD], lhsT=ones[:], rhs=bvr[:],
                                         start=False, stop=True)
                    vdst = vP[st][:, :].rearrange(
                        "p (h c) -> p h c", c=dh + 1)[:, :, 0:dh]
                    vsrc = pv[:, :HD].rearrange(
                        "p (h c) -> p h c", c=dh)[:, :, :]
                    nc.vector.tensor_scalar(vdst, vsrc, mkf[:, st:st + 1],
                                            None, op0=Mult)

                def qk_chunk(g, which, c):
                    dst, w_s, bcol = ((qT, wq_s, bqc) if which == 0
                                      else (kT, wk_s, bkc))
                    pq = pp.tile([128, CH], f32, name="pq", tag="pp")
                    for dt in range(DT):
                        nc.tensor.matmul(
                            pq[:],
                            lhsT=w_s[dt][:, g * 128:(g + 1) * 128],
                            rhs=xT_s[dt][:, c * CH:(c + 1) * CH],
                            start=(dt == 0), stop=(dt == DT - 1))
                    nc.vector.tensor_scalar_add(
                        dst[g][:, c * CH:(c + 1) * CH],
                        pq[:], bcol[:, g:g + 1])

                def final_chunk(qt, c):
                    pf = pp.tile([128, CH], f32, name="pf", tag="pp")
                    for wt in range(WOT):
                        nc.tensor.matmul(
                            pf[:],
                            lhsT=oT[wt][:, qt * 128:(qt + 1) * 128],
                            rhs=wo_s[wt][:, c * CH:(c + 1) * CH],
                            start=(wt == 0), stop=(wt == WOT - 1))
                    ys = ysb.tile([128, CH], f32, name="ys", tag="ys")
                    nc.vector.tensor_copy(ys[:], pf[:])
                    nc.sync.dma_start(
                        y_d[qt * 128:(qt + 1) * 128,
                            c * CH:(c + 1) * CH], ys[:])

                def attention_pair(g, p, smear=(), jit_vproj=False,
                                   all_act=False):
                    q0 = p * QHW
                    hA, hB = 2 * g, 2 * g + 1
                    ew = QHW // 4 if probe == "smallexp" else QHW
                    sm = list(smear)
                    si = 0
                    steps = (set(range(0, KT, 2)) if len(sm) > 2 else {4, 12})
                    ops = {}
                    pend = None

                    def emit_pv_half(kt, pt_pair, key, h, half):
                        pt, cast = pt_pair[0 if key == "A" else 1]
                        r0, r1 = (0, 64) if half == 0 else (64, 128)
                        if probe == "smallpv" and kt not in (0, KT - 1):
                            return
                        rhs = pt[r0:r1, 0:CH]
                        if cast:
                            rhs = rhs.bitcast(bf16)
                        nc.tensor.matmul(
                            ops[key + str(half)][:],
                            lhsT=vP[kt][r0:r1, h * 65:(h + 1) * 65],
                            rhs=rhs,
                            start=(kt == 0), stop=(kt == KT - 1))

                    def emit_exp(spt, act):
                        if act:
                            pt = ptp.tile([128, QHW], bf16, name="pt",
                                          tag="pta")
                            nc.scalar.activation(pt[:, :ew], spt[:, :ew], Exp)
                            return (pt, False)
                        pt = ptp.tile([128, QHW], i16, name="pti", tag="ptbi")
                        nc.vector.tensor_scalar(pt[:, :ew], spt[:, :ew],
                                                SCH_A, SCH_C,
                                                op0=Mult, op1=Add)
                        return (pt, True)

                    for kt in range(KT):
                        if jit_vproj:
                            v_proj(kt)
                        if si < len(sm) and kt in steps:
                            sm[si]()
                            si += 1
                        spA = pss.tile([128, QHW], f32, name="spA", tag="sp")
                        nc.tensor.matmul(
                            spA[:], lhsT=kT[g][0:64, kt * 128:(kt + 1) * 128],
                            rhs=qT[g][0:64, q0:q0 + QHW],
                            start=True, stop=True)
                        if pend is not None:
                            if kt == 1:
                                for tg in ("A0", "A1", "B0", "B1"):
                                    ops[tg] = pso.tile([65, QHW], f32,
                                                       name="o" + tg, tag=tg)
                            emit_pv_half(kt - 1, pend, "A", hA, 1)
                            emit_pv_half(kt - 1, pend, "A", hA, 0)
                        spB = pss.tile([128, QHW], f32, name="spB", tag="sp")
                        nc.tensor.matmul(
                            spB[:], lhsT=kT[g][64:128, kt * 128:(kt + 1) * 128],
                            rhs=qT[g][64:128, q0:q0 + QHW],
                            start=True, stop=True)
                        if pend is not None:
                            emit_pv_half(kt - 1, pend, "B", hB, 0)
                            emit_pv_half(kt - 1, pend, "B", hB, 1)
                        a_act = all_act or (kt not in A_DVE_KTS)
                        b_act = all_act
                        pend = (emit_exp(spA, a_act), emit_exp(spB, b_act))
                    while si < len(sm):
                        sm[si]()
                        si += 1
                    emit_pv_half(KT - 1, pend, "A", hA, 1)
                    emit_pv_half(KT - 1, pend, "A", hA, 0)
                    emit_pv_half(KT - 1, pend, "B", hB, 0)
                    emit_pv_half(KT - 1, pend, "B", hB, 1)
                    # normalize: merge lo+hi (copy then add; one PSUM operand
                    # per instruction), recip/broadcast/mult on Pool
                    for key, off in (("A", 0), ("B", 64)):
                        stg = nrm.tile([65, QHW], f32, name="stg",
                                       tag=f"stg{key}")
                        nc.vector.tensor_copy(stg[:], ops[key + "0"][:])
                        nc.vector.tensor_tensor(stg[:], stg[:],
                                                ops[key + "1"][:], op=Add)
                        rr = nrm.tile([1, QHW], f32, name="rr", tag=f"rr{key}")
                        nc.vector.reciprocal(rr[:], stg[64:65, :])
                        bc = nrm.tile([64, QHW], f32, name="bc", tag=f"bc{key}")
                        nc.gpsimd.partition_broadcast(bc[:], rr[:])
                        nc.gpsimd.tensor_tensor(
                            oT[g][off:off + 64, q0:q0 + QHW],
                            stg[0:64, :], bc[:], op=Mult)

                # ---- schedule ----
                for c in range(QCX):
                    for which in (0, 1):
                        qk_chunk(0, which, c)

                final_ready = []
                for p in range(NP):
                    for g in range(G):
                        smear = []
                        if p == 0:
                            if g < G - 1:
                                smear = [(lambda gg=g + 1, w=w, c=c:
                                          qk_chunk(gg, w, c))
                                         for c in range(QCX) for w in (0, 1)]
                        else:
                            take = min(2, len(final_ready))
                            smear = final_ready[:take]
                            final_ready = final_ready[take:]
                        attention_pair(g, p, smear=smear,
                                       jit_vproj=(p == 0 and g == 0),
                                       all_act=(p == 0 and g < G - 1))
                    final_ready.extend(
                        (lambda qt=qt, c=c: final_chunk(qt, c))
                        for qt in range((QHW * p) // 128,
                                        (QHW * (p + 1)) // 128)
                        for c in range(DCH))
                for u in final_ready:
                    u()

    nc.compile()
    return nc


def pack_xT(xt, S, D):
    """[D, S] -> contiguous [DT*QCX*128, CH] blocks matching build_nc's DMAs."""
    CH = min(512, S)
    DT, QCX = D // 128, S // CH
    return np.ascontiguousarray(
        xt.reshape(DT, 128, QCX, CH).transpose(0, 2, 1, 3)
    ).reshape(DT * QCX * 128, CH)


def host_shard(x, mask, Wq, bq, Wk, bk, Wv, bv, Wo, bo,
               S=S, D=D, HC=HC, dh=DH):
    """Build the 8 per-core input maps (host-side layout prep)."""
    KT = S // 128
    G = HC // 2
    HD = HC * dh
    scale = 1.0 / np.sqrt(dh)
    in_maps = []
    x = np.asarray(x, np.float32)
    mask = np.asarray(mask)
    for c in range(N_CORES):
        b = c // 2
        hs = (c % 2) * HD
        cols = slice(hs, hs + HD)
        m = 1.0 - mask[b].astype(np.float32)
        in_maps.append({
            "xT": pack_xT(np.ascontiguousarray(x[b].T), S, D).astype(BF16),
            "wq": (np.asarray(Wq)[:, cols] * scale).astype(BF16),
            "wk": np.asarray(Wk)[:, cols].astype(BF16),
            "wv": np.asarray(Wv)[:, cols].astype(BF16),
            "wo": np.asarray(Wo)[cols, :].astype(BF16),
            "mkf": np.ascontiguousarray(m.reshape(KT, 128).T),
            "bqc": np.ascontiguousarray(
                (np.asarray(bq, np.float32)[cols] * scale).reshape(G, 128).T),
            "bkc": np.ascontiguousarray(
                np.asarray(bk, np.float32)[cols].reshape(G, 128).T),
            "bvr": np.asarray(bv, np.float32)[cols].reshape(1, HD).astype(BF16),
        })
    return in_maps


def host_gather(results, bo, B=B, S=S, D=D):
    out = np.empty((B, S, D), np.float32)
    bo = np.asarray(bo, np.float32)
    for b in range(B):
        out[b] = results[2 * b]["y"] + results[2 * b + 1]["y"] + bo
    return out


_NC_CACHE = {}


def kernel(x, mask, Wq, bq, Wk, bk, Wv, bv, Wo, bo):
    from concourse.bass_utils import run_bass_kernel_spmd
    with_bv = bool(np.any(np.asarray(bv)))
    if with_bv not in _NC_CACHE:
        _NC_CACHE[with_bv] = build_nc(with_bv=with_bv)
    in_maps = host_shard(x, mask, Wq, bq, Wk, bk, Wv, bv, Wo, bo)
    res = run_bass_kernel_spmd(_NC_CACHE[with_bv], in_maps,
                               core_ids=list(range(N_CORES)))
    return host_gather(res.results, bo)
